# revision 22
# baseline (speedup 1.0000x reference)
"""Trn2 Bass kernel for nn_Attention_16793322128104.

Sharding: 8 cores = 2 batches x 4 head-groups (4 heads each).
Per core: fp16 QKV projection (768 Wqkv cols), then per head a SINGLE
S = Q.K^T pass in [q, k] orientation: per-query max via vector
reduce_max (free axis), exp on the scalar engine with the -8*max shift
as a per-partition activation bias, P transposed to [k, q] by the DMA
XBAR transpose unit, AV with a ones-column denominator, deferred
normalization, fp16 partial out-projection (host sums 4 partials).

Pipeline: head h's S/exp/transpose overlaps AV of head h-1 (PT planes
ping-pong) and, for h0, the QKV v-wave. PSUM: 3 rotating [128,1024]
stream slots + a 2-bank AV accumulator.
"""

import sys
from contextlib import ExitStack

import numpy as np

sys.path.insert(0, "/opt/trn_rl_repo")

import concourse.bass as bass
import concourse.bacc as bacc
import concourse.mybir as mybir
from concourse import tile
from concourse.bass_utils import run_bass_kernel_spmd

F32 = mybir.dt.float32
F16 = mybir.dt.float16
IDENT = mybir.ActivationFunctionType.Identity
EXP = mybir.ActivationFunctionType.Exp
XAX = mybir.AxisListType.X

N_TOK = 2048
DIM = 1024
NH = 4                # heads per core
DH = 64               # head dim
SCALE = 8.0           # sqrt(DH); reference MULTIPLIES by sqrt(d_head)

_CACHE = {}


def build_nc():
    nc = bacc.Bacc()
    xt_d = nc.declare_dram_parameter("xt", [DIM + 1, N_TOK], F16, isOutput=False)
    wg_d = nc.declare_dram_parameter("wg", [DIM + 1, 3 * NH * DH], F16, isOutput=False)
    qkb_d = nc.declare_dram_parameter("qkb", [128, 4], F32, isOutput=False)
    wo_d = nc.declare_dram_parameter("wout", [2 * 128, DIM], F16, isOutput=False)
    on16_d = nc.declare_dram_parameter("ones16", [1, DH], F16, isOutput=False)
    out_d = nc.declare_dram_parameter("out", [DIM, N_TOK], F16, isOutput=True)

    with ExitStack() as ctx:
        tc = ctx.enter_context(tile.TileContext(nc))
        pers = ctx.enter_context(tc.tile_pool(name="pers", bufs=1))
        ps = ctx.enter_context(
            tc.tile_pool(name="ps", bufs=3, space=bass.MemorySpace.PSUM)
        )
        pop = ctx.enter_context(
            tc.tile_pool(name="pop", bufs=1, space=bass.MemorySpace.PSUM)
        )

        q2 = [pers.tile([DH, N_TOK], F16, tag=f"q2{h}", name=f"q2{h}") for h in range(NH)]
        k2 = [pers.tile([DH, N_TOK], F16, tag=f"k2{h}", name=f"k2{h}") for h in range(NH)]
        vsb = [pers.tile([128, NH, DH + 1], F16, tag=f"v{m}", name=f"v{m}") for m in range(16)]
        o2 = [pers.tile([128, N_TOK], F16, tag=f"o2{t}", name=f"o2{t}") for t in range(2)]
        wo_sb = [pers.tile([128, DIM], F16, tag=f"wo{t}", name=f"wo{t}") for t in range(2)]
        qkb = pers.tile([128, 4], F32, tag="qkb", name="qkbsb")
        on16 = pers.tile([1, DH], F16, tag="on16", name="on16sb")
        mc2 = [pers.tile([128, 16, 2], F32, tag=f"mc{h}", name=f"mc{h}") for h in range(NH)]
        mcf = [pers.tile([128, 16], F32, tag=f"mcf{h}", name=f"mcf{h}") for h in range(NH)]
        negb = [pers.tile([128, 16], F32, tag=f"nb{h}", name=f"nb{h}") for h in range(NH)]
        # deferred-normalization temporaries
        denall = pers.tile([8, N_TOK // 2], F16, tag="denall", name="denall")
        dtmp = [pers.tile([1, N_TOK // 2], F16, tag="dtmp0", name="dtmp0")] * 2
        dnf = [pers.tile([128, 8], F16, tag=f"dnf{p}", name=f"dnf{p}") for p in range(2)]
        dnr = [pers.tile([128, 8], F32, tag=f"dnr{p}", name=f"dnr{p}") for p in range(2)]
        dnh = [pers.tile([128, 8], F16, tag=f"dnh{p}", name=f"dnh{p}") for p in range(2)]
        rrj = [pers.tile([1, N_TOK // 2], F16, tag=f"rr{p}", name=f"rr{p}") for p in range(2)]
        rm = pers.tile([128, N_TOK // 2], F16, tag="rm", name="rmsb")
        # PT plane 0 lives from the start (so head 0's transposes can run
        # during phase A); plane 1 opens after the xt/wg pools close.
        PT0 = pers.tile([128, 16, N_TOK], F16, tag="PT0", name="PT0sb")
        # P staging tiles feeding the DMA transpose
        pp_pool = ctx.enter_context(tc.tile_pool(name="Pp", bufs=2))

        nc.sync.dma_start(qkb[:], qkb_d[:])
        nc.sync.dma_start(on16[:], on16_d[:])
        for m in range(16):
            nc.vector.memset(vsb[m][:, :, DH : DH + 1], 1.0)
        for t in range(2):
            nc.sync.dma_start(wo_sb[t][:], wo_d[t * 128 : (t + 1) * 128, :])

        PTp = [PT0, None]  # plane 1 assigned after phase A

        # ---------------- per-head step: S -> max -> exp -> transpose ------
        def s_step(h, qt):
            sts = []
            for half in range(2):
                st = ps.tile([128, N_TOK // 2], F32, tag="s", name="st")
                for j2 in range(2):
                    nc.tensor.matmul(
                        st[:, j2 * 512 : (j2 + 1) * 512],
                        q2[h][:, qt * 128 : (qt + 1) * 128],
                        k2[h][:, half * 1024 + j2 * 512 : half * 1024 + (j2 + 1) * 512],
                        start=True,
                        stop=True,
                    )
                nc.vector.reduce_max(mc2[h][:, qt, half : half + 1], st[:], axis=XAX)
                sts.append(st)
            nc.vector.reduce_max(mcf[h][:, qt : qt + 1], mc2[h][:, qt, :], axis=XAX)
            nc.vector.tensor_scalar_mul(
                negb[h][:, qt : qt + 1], mcf[h][:, qt : qt + 1], -SCALE
            )
            P = pp_pool.tile([128, N_TOK], F16, tag="P", name="P")
            for half in range(2):
                nc.scalar.activation(
                    P[:, half * 1024 : (half + 1) * 1024], sts[half][:],
                    EXP, scale=SCALE, bias=negb[h][:, qt : qt + 1],
                )
            nc.sync.dma_start_transpose(
                PTp[h % 2][:, :, qt * 128 : (qt + 1) * 128], P[:]
            )

        # ---------------- AV + deferred normalization helpers --------------
        def av_j_mm(h, j, m2, po):
            for u in range(2):
                nc.tensor.matmul(
                    po[:, u * 512 : (u + 1) * 512],
                    vsb[m2][:, h, :],
                    PTp[h % 2][:, m2, j * 1024 + u * 512 : j * 1024 + (u + 1) * 512],
                    start=(m2 == 0),
                    stop=(m2 == 15),
                )

        def o2q(h, j):
            return o2[h // 2][(h % 2) * DH : (h % 2) * DH + DH, j * 1024 : (j + 1) * 1024]

        def stash_j(h, j, po):
            # raw (unnormalized) AV output into o2; denominator row stashed
            # via a partition-0 ping tile. Normalized in a batched tail pass.
            idx = 2 * h + j
            p = idx % 2
            nc.vector.tensor_copy(o2q(h, j), po[0:DH, :])
            nc.scalar.copy(dtmp[p][:], po[DH : DH + 1, :])
            nc.sync.dma_start(denall[idx : idx + 1, :], dtmp[p][:])

        def norm_j(h, j):
            p = j % 2
            idx = 2 * h + j
            nc.sync.dma_start(dnf[p][:], denall[idx : idx + 1, :])
            nc.vector.reciprocal(dnr[p][:], dnf[p][:])
            nc.vector.tensor_copy(dnh[p][:], dnr[p][:])
            nc.sync.dma_start(rrj[p][:], dnh[p][:])
            pr = ps.tile([128, N_TOK // 2], F32, tag="s", name="pr")
            for u in range(2):
                nc.tensor.matmul(
                    pr[0:DH, u * 512 : (u + 1) * 512],
                    on16[:], rrj[p][:, u * 512 : (u + 1) * 512],
                    start=True, stop=True,
                )
            rows = slice((h % 2) * DH, (h % 2) * DH + DH)
            nc.scalar.copy(rm[rows, :], pr[0:DH, :])
            nc.vector.tensor_mul(o2q(h, j), o2q(h, j), rm[rows, :])

        def av_step(h, m, po_box):
            # two AV m2-tiles of head h at pipeline step m (j = m // 8)
            j = m // 8
            if m % 8 == 0:
                if m == 8:
                    stash_j(h, 0, po_box[0])
                po_box[0] = pop.tile([DH + 1, N_TOK // 2], F32, tag="po", name="po")
            av_j_mm(h, j, 2 * (m % 8), po_box[0])
            av_j_mm(h, j, 2 * (m % 8) + 1, po_box[0])

        # ---------------- phase A: QKV projection (fp16) -------------------
        with (
            tc.tile_pool(name="xt", bufs=1) as xt_pool,
            tc.tile_pool(name="wgp", bufs=1) as wg_pool,
        ):
            xt_sb = xt_pool.tile([128, 8, N_TOK], F16, tag="xta", name="xta")
            xtr = xt_pool.tile([1, N_TOK], F16, tag="xtr", name="xtr")
            wg_sb = wg_pool.tile([128, 8, 3 * NH * DH], F16, tag="wga", name="wga")
            wgr = wg_pool.tile([1, 3 * NH * DH], F16, tag="wgr", name="wgr")
            for c in range(8):
                nc.sync.dma_start(wg_sb[:, c, :], wg_d[c * 128 : (c + 1) * 128, :])
                nc.sync.dma_start(xt_sb[:, c, :], xt_d[c * 128 : (c + 1) * 128, :])
            nc.sync.dma_start(xtr[:], xt_d[DIM : DIM + 1, :])
            nc.sync.dma_start(wgr[:], wg_d[DIM : DIM + 1, :])

            # q/k waves: ft 0/1 = q heads01/23 (cols 0:256), 2/3 = k (256:512)
            def qk_wave(ft):
                dst = q2 if ft < 2 else k2
                hb = 2 * (ft % 2)
                pw = [ps.tile([128, N_TOK // 2], F32, tag="s", name="pw") for _ in range(2)]
                for c in range(8):
                    for half in range(2):
                        for tj in range(2):
                            nc.tensor.matmul(
                                pw[half][:, tj * 512 : (tj + 1) * 512],
                                wg_sb[:, c, ft * 128 : (ft + 1) * 128],
                                xt_sb[:, c, half * 1024 + tj * 512 : half * 1024 + (tj + 1) * 512],
                                start=(c == 0),
                                stop=(c == 7),
                            )
                for half in range(2):
                    cols = slice(half * 1024, (half + 1) * 1024)
                    nc.scalar.activation(
                        dst[hb][:, cols], pw[half][0:DH, :], IDENT,
                        bias=qkb[0:DH, ft : ft + 1],
                    )
                    nc.scalar.activation(
                        dst[hb + 1][:, cols], pw[half][DH:128, :], IDENT,
                        bias=qkb[DH:128, ft : ft + 1],
                    )

            def v_tile(i):
                # two token-tiles (2i, 2i+1) of v into one psum tile
                pv = ps.tile([128, N_TOK // 2], F32, tag="s", name="pv")
                for c in range(9):
                    for u in range(2):
                        tt = 2 * i + u
                        lhs = (
                            xt_sb[:, c, tt * 128 : (tt + 1) * 128]
                            if c < 8
                            else xtr[:, tt * 128 : (tt + 1) * 128]
                        )
                        rhs = (
                            wg_sb[:, c, 2 * NH * DH : 3 * NH * DH]
                            if c < 8
                            else wgr[:, 2 * NH * DH : 3 * NH * DH]
                        )
                        nc.tensor.matmul(
                            pv[:, u * 512 : u * 512 + NH * DH],
                            lhs, rhs,
                            start=(c == 0),
                            stop=(c == 8),
                        )
                for u in range(2):
                    tt = 2 * i + u
                    nc.scalar.copy(
                        vsb[tt][:, :, 0:DH],
                        pv[:, u * 512 : u * 512 + NH * DH].rearrange(
                            "p (h d) -> p h d", h=NH
                        ),
                    )

            qk_wave(0)          # q heads 0,1
            qk_wave(2)          # k heads 0,1
            # head 0's S/exp/transpose pipeline interleaves with the rest
            qk_wave(1)
            s_step(0, 0)
            qk_wave(3)
            s_step(0, 1)
            for i in range(8):
                v_tile(i)
                if i < 7:
                    s_step(0, 2 + 2 * i)
                    s_step(0, 3 + 2 * i)

        # ---------------- attention pipeline -------------------------------
        ptp1 = ctx.enter_context(tc.tile_pool(name="ptp1", bufs=1))
        PTp[1] = ptp1.tile([128, 16, N_TOK], F16, tag="PT1", name="PT1sb")

        # pp(h): S/exp/transpose of head h + AV of head h-1
        for h in range(1, NH):
            po_box = [None]
            for m in range(16):
                av_step(h - 1, m, po_box)
                s_step(h, m)
            stash_j(h - 1, 1, po_box[0])

        # tail: AV for head 3 with earlier heads' norms interleaved
        norms = [(h, j) for h in range(3) for j in range(2)]
        ni = 0
        po_box = [None]
        for m in range(16):
            av_step(3, m, po_box)
            if m % 3 == 2 and ni < len(norms):
                norm_j(*norms[ni])
                ni += 1
        stash_j(3, 1, po_box[0])
        while ni < len(norms):
            norm_j(*norms[ni])
            ni += 1
        norm_j(3, 0)
        norm_j(3, 1)

        # ---------------- out projection -----------------------------------
        with tc.tile_pool(name="ob", bufs=2) as obp:
            for dc in range(8):
                for half in range(2):
                    pout = ps.tile([128, N_TOK // 2], F32, tag="s", name="pout")
                    for ht in range(2):
                        for u in range(2):
                            nc.tensor.matmul(
                                pout[:, u * 512 : (u + 1) * 512],
                                wo_sb[ht][:, dc * 128 : (dc + 1) * 128],
                                o2[ht][:, half * 1024 + u * 512 : half * 1024 + (u + 1) * 512],
                                start=(ht == 0),
                                stop=(ht == 1),
                            )
                    ob = obp.tile([128, N_TOK // 2], F16, tag="ob", name="ob")
                    if (2 * dc + half) % 2 == 0:
                        nc.scalar.copy(ob[:], pout[:])
                    else:
                        nc.vector.tensor_copy(ob[:], pout[:])
                    nc.sync.dma_start(
                        out_d[dc * 128 : (dc + 1) * 128, half * 1024 : (half + 1) * 1024],
                        ob[:],
                    )
    nc.finalize()
    return nc


def _get_nc():
    if "nc" not in _CACHE:
        _CACHE["nc"] = build_nc()
    return _CACHE["nc"]


def kernel(x, Wqkv, bqkv, Wout, bout):
    x = np.asarray(x, np.float32)
    Wqkv = np.asarray(Wqkv, np.float32)
    bqkv = np.asarray(bqkv, np.float32)
    Wout = np.asarray(Wout, np.float32)
    bout = np.asarray(bout, np.float32)
    B = x.shape[0]
    ones_row16 = np.ones((1, N_TOK), np.float16)

    in_maps = []
    for c in range(8):
        b, g = c // 4, c % 4
        xt = np.concatenate(
            [np.ascontiguousarray(x[b].T).astype(np.float16), ones_row16], 0
        )
        cols, bias = [], []
        for blk in range(3):  # q, k, v column blocks of Wqkv
            s = blk * DIM + g * NH * DH
            cols.append(Wqkv[:, s : s + NH * DH])
            bias.append(bqkv[s : s + NH * DH])
        wg = np.concatenate(
            [np.concatenate(cols, 1), np.concatenate(bias)[None, :]], 0
        ).astype(np.float16)
        qb = bqkv[g * 256 : (g + 1) * 256]
        kb = bqkv[DIM + g * 256 : DIM + (g + 1) * 256]
        qkb = np.stack([qb[:128], qb[128:], kb[:128], kb[128:]], 1).astype(np.float32)
        wo = np.ascontiguousarray(Wout[g * NH * DH : (g + 1) * NH * DH, :]).astype(
            np.float16
        )
        in_maps.append(
            {
                "xt": np.ascontiguousarray(xt),
                "wg": np.ascontiguousarray(wg),
                "qkb": np.ascontiguousarray(qkb),
                "wout": wo,
                "ones16": np.ones((1, DH), np.float16),
            }
        )

    _CACHE["last_in_maps"] = in_maps
    res = run_bass_kernel_spmd(_get_nc(), in_maps, list(range(8))).results
    out = np.empty((B, N_TOK, DIM), np.float32)
    for b in range(B):
        acc = res[4 * b]["out"].astype(np.float32)
        for g in range(1, 4):
            acc = acc + res[4 * b + g]["out"].astype(np.float32)
        out[b] = acc.T + bout[None, :]
    return out


if __name__ == "__main__":
    rng = np.random.default_rng(0)
    x = rng.standard_normal((2, N_TOK, DIM)).astype(np.float32)
    Wqkv = (rng.standard_normal((DIM, 3 * DIM)) * DIM**-0.5).astype(np.float32)
    bqkv = (rng.standard_normal(3 * DIM) * 0.02).astype(np.float32)
    Wout = (rng.standard_normal((DIM, DIM)) * DIM**-0.5).astype(np.float32)
    bout = (rng.standard_normal(DIM) * 0.02).astype(np.float32)
    o = kernel(x=x, Wqkv=Wqkv, bqkv=bqkv, Wout=Wout, bout=bout)
    print("kernel ran, out shape", o.shape)


# revision 23
# speedup vs baseline: 1.1536x; 1.1536x over previous
"""Trn2 Bass kernel for nn_Attention_16793322128104.

Sharding: 8 cores = 2 batches x 4 head-groups (4 heads each).
Per core: fp16 QKV projection (768 Wqkv cols), 4 attention heads with
exact per-query max (pass-1 fp16 S + vector reduce_max), softmax shift
folded into the S^T matmul as a 65th contraction row (fp16), exp on the
scalar engine into an fp16 PT ring, AV with ones-column denominator,
deferred batched normalization, fp16 partial out-projection (host sums
the 4 head-group partials per batch).

Pipeline: head h's S^T/exp overlaps AV of head h-1 (PT ring, 24 slots)
and pass-1 of head h+1. PSUM: 3 rotating [128,1024] stream slots +
a 2-bank AV accumulator.
"""

import sys
from contextlib import ExitStack

import numpy as np

sys.path.insert(0, "/opt/trn_rl_repo")

import concourse.bass as bass
import concourse.bacc as bacc
import concourse.mybir as mybir
from concourse import tile
from concourse.bass_utils import run_bass_kernel_spmd

F32 = mybir.dt.float32
F16 = mybir.dt.float16
IDENT = mybir.ActivationFunctionType.Identity
EXP = mybir.ActivationFunctionType.Exp
XAX = mybir.AxisListType.X

N_TOK = 2048
DIM = 1024
NH = 4                # heads per core
DH = 64               # head dim
SCALE = 8.0           # sqrt(DH); reference MULTIPLIES by sqrt(d_head)
RING = 24             # PT ring slots (16 per head, AV trails one head)

_CACHE = {}


def _rs(h, m):
    return (16 * h + m) % RING


def build_nc():
    nc = bacc.Bacc()
    xt_d = nc.declare_dram_parameter("xt", [DIM + 1, N_TOK], F16, isOutput=False)
    wg_d = nc.declare_dram_parameter("wg", [DIM + 1, 3 * NH * DH], F16, isOutput=False)
    qkb_d = nc.declare_dram_parameter("qkb", [128, 4], F32, isOutput=False)
    wo_d = nc.declare_dram_parameter("wout", [2 * 128, DIM], F16, isOutput=False)
    id_d = nc.declare_dram_parameter("ident", [128, 128], F32, isOutput=False)
    on16_d = nc.declare_dram_parameter("ones16", [1, DH], F16, isOutput=False)
    out_d = nc.declare_dram_parameter("out", [DIM, N_TOK], F16, isOutput=True)

    with ExitStack() as ctx:
        tc = ctx.enter_context(tile.TileContext(nc))
        pers = ctx.enter_context(tc.tile_pool(name="pers", bufs=1))
        ps = ctx.enter_context(
            tc.tile_pool(name="ps", bufs=3, space=bass.MemorySpace.PSUM)
        )
        pop = ctx.enter_context(
            tc.tile_pool(name="pop", bufs=1, space=bass.MemorySpace.PSUM)
        )

        q2 = [pers.tile([DH + 1, N_TOK], F16, tag=f"q2{h}", name=f"q2{h}") for h in range(NH)]
        k2 = [pers.tile([DH + 1, N_TOK], F16, tag=f"k2{h}", name=f"k2{h}") for h in range(NH)]
        vsb = [pers.tile([128, NH, DH + 1], F16, tag=f"v{m}", name=f"v{m}") for m in range(16)]
        o2 = [pers.tile([128, N_TOK], F16, tag=f"o2{t}", name=f"o2{t}") for t in range(2)]
        wo_sb = [pers.tile([128, DIM], F16, tag=f"wo{t}", name=f"wo{t}") for t in range(2)]
        ident = pers.tile([128, 128], F32, tag="id", name="identsb")
        qkb = pers.tile([128, 4], F32, tag="qkb", name="qkbsb")
        on16 = pers.tile([1, DH], F16, tag="on16", name="on16sb")
        mc2 = [pers.tile([128, 16, 2], F32, tag=f"mc{h}", name=f"mc{h}") for h in range(NH)]
        mcf = [pers.tile([128, 16], F32, tag=f"mcf{h}", name=f"mcf{h}") for h in range(NH)]
        negm = [
            [pers.tile([8, 128], F16, tag=f"nm{h}{u}", name=f"nm{h}{u}") for u in range(2)]
            for h in range(NH)
        ]
        # deferred-normalization temporaries
        denall = pers.tile([8, N_TOK // 2], F16, tag="denall", name="denall")
        dtmp = [pers.tile([1, N_TOK // 2], F16, tag=f"dtmp{p}", name=f"dtmp{p}") for p in range(2)]
        dnf = [pers.tile([128, 8], F16, tag=f"dnf{p}", name=f"dnf{p}") for p in range(2)]
        dnr = [pers.tile([128, 8], F32, tag=f"dnr{p}", name=f"dnr{p}") for p in range(2)]
        dnh = [pers.tile([128, 8], F16, tag=f"dnh{p}", name=f"dnh{p}") for p in range(2)]
        rrj = [pers.tile([1, N_TOK // 2], F16, tag=f"rr{p}", name=f"rr{p}") for p in range(2)]
        rm = pers.tile([128, N_TOK // 2], F16, tag="rm", name="rmsb")

        nc.sync.dma_start(ident[:], id_d[:])
        nc.sync.dma_start(qkb[:], qkb_d[:])
        nc.sync.dma_start(on16[:], on16_d[:])
        for h in range(NH):
            nc.sync.dma_start(k2[h][DH : DH + 1, :], xt_d[DIM : DIM + 1, :])
        for m in range(16):
            nc.vector.memset(vsb[m][:, :, DH : DH + 1], 1.0)
        for t in range(2):
            nc.sync.dma_start(wo_sb[t][:], wo_d[t * 128 : (t + 1) * 128, :])

        # ---------------- pass-1 helper: per-query max for head h ----------
        def negm_half(h, half):
            # fold maxes for qt block [8*half, 8*half+8) into q2 row 64
            qs = slice(8 * half, 8 * half + 8)
            nc.vector.reduce_max(mcf[h][:, qs], mc2[h][:, qs, :], axis=XAX)
            pst = ps.tile([128, N_TOK // 2], F32, tag="s", name="pst")
            nc.tensor.transpose(pst[0:8, 0:128], mcf[h][:, qs], ident[:])
            nc.vector.tensor_scalar_mul(negm[h][half][:], pst[0:8, 0:128], -1.0)
            nc.sync.dma_start(
                q2[h][DH : DH + 1, half * 1024 : (half + 1) * 1024], negm[h][half][:]
            )

        def p1_qt(h, qt):
            for half in range(2):
                p = ps.tile([128, N_TOK // 2], F32, tag="s", name="p1")
                for kc in range(2):
                    nc.tensor.matmul(
                        p[:, kc * 512 : (kc + 1) * 512],
                        q2[h][0:DH, qt * 128 : (qt + 1) * 128],
                        k2[h][0:DH, half * 1024 + kc * 512 : half * 1024 + (kc + 1) * 512],
                        start=True,
                        stop=True,
                    )
                nc.vector.reduce_max(mc2[h][:, qt, half : half + 1], p[:], axis=XAX)
            if qt == 8:
                negm_half(h, 0)
            if qt == 15:
                negm_half(h, 1)

        # ---------------- AV + deferred normalization helpers --------------
        def av_j_mm(h, j, m2, po):
            for u in range(2):
                nc.tensor.matmul(
                    po[:, u * 512 : (u + 1) * 512],
                    vsb[m2][:, h, :],
                    PT[:, _rs(h, m2), j * 1024 + u * 512 : j * 1024 + (u + 1) * 512],
                    start=(m2 == 0),
                    stop=(m2 == 15),
                )

        def o2q(h, j):
            return o2[h // 2][(h % 2) * DH : (h % 2) * DH + DH, j * 1024 : (j + 1) * 1024]

        def stash_j(h, j, po):
            # raw (unnormalized) AV output into o2; denominator row stashed
            # via a partition-0 ping tile. Normalized in a batched tail pass.
            idx = 2 * h + j
            p = idx % 2
            nc.vector.tensor_copy(o2q(h, j), po[0:DH, :])
            nc.scalar.copy(dtmp[p][:], po[DH : DH + 1, :])
            nc.sync.dma_start(denall[idx : idx + 1, :], dtmp[p][:])

        def norm_j(h, j):
            p = j % 2
            idx = 2 * h + j
            nc.sync.dma_start(dnf[p][:], denall[idx : idx + 1, :])
            nc.vector.reciprocal(dnr[p][:], dnf[p][:])
            nc.vector.tensor_copy(dnh[p][:], dnr[p][:])
            nc.sync.dma_start(rrj[p][:], dnh[p][:])
            pr = ps.tile([128, N_TOK // 2], F32, tag="s", name="pr")
            for u in range(2):
                nc.tensor.matmul(
                    pr[0:DH, u * 512 : (u + 1) * 512],
                    on16[:], rrj[p][:, u * 512 : (u + 1) * 512],
                    start=True, stop=True,
                )
            rows = slice((h % 2) * DH, (h % 2) * DH + DH)
            nc.scalar.copy(rm[rows, :], pr[0:DH, :])
            nc.vector.tensor_mul(o2q(h, j), o2q(h, j), rm[rows, :])

        def av_step(h, m, po_box):
            # two AV m2-tiles of head h at pipeline step m (j = m // 8)
            j = m // 8
            if m % 8 == 0:
                if m == 8:
                    stash_j(h, 0, po_box[0])
                po_box[0] = pop.tile([DH + 1, N_TOK // 2], F32, tag="po", name="po")
            av_j_mm(h, j, 2 * (m % 8), po_box[0])
            av_j_mm(h, j, 2 * (m % 8) + 1, po_box[0])

        # ---------------- phase A: QKV projection (fp16) -------------------
        with (
            tc.tile_pool(name="xt", bufs=1) as xt_pool,
            tc.tile_pool(name="wgp", bufs=1) as wg_pool,
        ):
            xt_sb = xt_pool.tile([128, 8, N_TOK], F16, tag="xta", name="xta")
            xtr = xt_pool.tile([1, N_TOK], F16, tag="xtr", name="xtr")
            wg_sb = wg_pool.tile([128, 8, 3 * NH * DH], F16, tag="wga", name="wga")
            wgr = wg_pool.tile([1, 3 * NH * DH], F16, tag="wgr", name="wgr")
            for c in range(8):
                nc.sync.dma_start(wg_sb[:, c, :], wg_d[c * 128 : (c + 1) * 128, :])
                nc.sync.dma_start(xt_sb[:, c, :], xt_d[c * 128 : (c + 1) * 128, :])
            nc.sync.dma_start(xtr[:], xt_d[DIM : DIM + 1, :])
            nc.sync.dma_start(wgr[:], wg_d[DIM : DIM + 1, :])

            # q/k waves: ft 0/1 = q heads01/23 (cols 0:256), 2/3 = k (256:512)
            def qk_wave(ft):
                dst = q2 if ft < 2 else k2
                hb = 2 * (ft % 2)
                pw = [ps.tile([128, N_TOK // 2], F32, tag="s", name="pw") for _ in range(2)]
                for c in range(8):
                    for half in range(2):
                        for tj in range(2):
                            nc.tensor.matmul(
                                pw[half][:, tj * 512 : (tj + 1) * 512],
                                wg_sb[:, c, ft * 128 : (ft + 1) * 128],
                                xt_sb[:, c, half * 1024 + tj * 512 : half * 1024 + (tj + 1) * 512],
                                start=(c == 0),
                                stop=(c == 7),
                            )
                for half in range(2):
                    cols = slice(half * 1024, (half + 1) * 1024)
                    nc.scalar.activation(
                        dst[hb][0:DH, cols], pw[half][0:DH, :], IDENT,
                        bias=qkb[0:DH, ft : ft + 1],
                    )
                    nc.scalar.activation(
                        dst[hb + 1][0:DH, cols], pw[half][DH:128, :], IDENT,
                        bias=qkb[DH:128, ft : ft + 1],
                    )

            def v_tile(i):
                # two token-tiles (2i, 2i+1) of v into one psum tile
                pv = ps.tile([128, N_TOK // 2], F32, tag="s", name="pv")
                for c in range(9):
                    for u in range(2):
                        tt = 2 * i + u
                        lhs = (
                            xt_sb[:, c, tt * 128 : (tt + 1) * 128]
                            if c < 8
                            else xtr[:, tt * 128 : (tt + 1) * 128]
                        )
                        rhs = (
                            wg_sb[:, c, 2 * NH * DH : 3 * NH * DH]
                            if c < 8
                            else wgr[:, 2 * NH * DH : 3 * NH * DH]
                        )
                        nc.tensor.matmul(
                            pv[:, u * 512 : u * 512 + NH * DH],
                            lhs, rhs,
                            start=(c == 0),
                            stop=(c == 8),
                        )
                for u in range(2):
                    tt = 2 * i + u
                    nc.scalar.copy(
                        vsb[tt][:, :, 0:DH],
                        pv[:, u * 512 : u * 512 + NH * DH].rearrange(
                            "p (h d) -> p h d", h=NH
                        ),
                    )

            qk_wave(0)          # q heads 0,1
            qk_wave(2)          # k heads 0,1
            # interleave q23/k23 waves and v tiles with pass-1 of head 0
            qk_wave(1)
            p1_qt(0, 0)
            qk_wave(3)
            p1_qt(0, 1)
            for i in range(8):
                v_tile(i)
                if i < 7:
                    p1_qt(0, 2 + 2 * i)
                    p1_qt(0, 3 + 2 * i)

        # ---------------- attention: S^T + exp, pipelined ------------------
        # PT ring lives in its own pool so it reuses the SBUF freed by the
        # xt/wg pools above.
        ptp = ctx.enter_context(tc.tile_pool(name="ptp", bufs=1))
        PT = ptp.tile([128, RING, N_TOK], F16, tag="PT", name="PTsb")

        def pp(h):
            hp = h + 1 if h < 3 else None     # pass-1 head piggybacked
            ha = h - 1 if h >= 1 else None    # AV head piggybacked
            po_box = [None]
            for m in range(16):
                # AV for h-1 first: exp(h, m) reuses the PT ring slot that
                # AV(h-1) reads at this step, so its readers must be emitted
                # ahead of the overwrite.
                if ha is not None:
                    av_step(ha, m, po_box)
                for half in range(2):
                    st = ps.tile([128, N_TOK // 2], F32, tag="s", name="st")
                    for j2 in range(2):
                        nc.tensor.matmul(
                            st[:, j2 * 512 : (j2 + 1) * 512],
                            k2[h][:, m * 128 : (m + 1) * 128],
                            q2[h][:, half * 1024 + j2 * 512 : half * 1024 + (j2 + 1) * 512],
                            start=True,
                            stop=True,
                        )
                    nc.scalar.activation(
                        PT[:, _rs(h, m), half * 1024 : (half + 1) * 1024],
                        st[:], EXP, scale=SCALE,
                    )
                if hp is not None:
                    p1_qt(hp, m)
            if ha is not None:
                stash_j(ha, 1, po_box[0])

        for h in range(NH):
            pp(h)

        # tail: AV for head 3 with earlier heads' norms interleaved
        norms = [(h, j) for h in range(3) for j in range(2)]
        ni = 0
        po_box = [None]
        for m in range(16):
            av_step(3, m, po_box)
            if m % 3 == 2 and ni < len(norms):
                norm_j(*norms[ni])
                ni += 1
        stash_j(3, 1, po_box[0])
        while ni < len(norms):
            norm_j(*norms[ni])
            ni += 1
        norm_j(3, 0)
        norm_j(3, 1)

        # ---------------- out projection -----------------------------------
        with tc.tile_pool(name="ob", bufs=3) as obp:
            for dc in range(8):
                for half in range(2):
                    pout = ps.tile([128, N_TOK // 2], F32, tag="s", name="pout")
                    for ht in range(2):
                        for u in range(2):
                            nc.tensor.matmul(
                                pout[:, u * 512 : (u + 1) * 512],
                                wo_sb[ht][:, dc * 128 : (dc + 1) * 128],
                                o2[ht][:, half * 1024 + u * 512 : half * 1024 + (u + 1) * 512],
                                start=(ht == 0),
                                stop=(ht == 1),
                            )
                    ob = obp.tile([128, N_TOK // 2], F16, tag="ob", name="ob")
                    if (2 * dc + half) % 2 == 0:
                        nc.scalar.copy(ob[:], pout[:])
                    else:
                        nc.vector.tensor_copy(ob[:], pout[:])
                    nc.sync.dma_start(
                        out_d[dc * 128 : (dc + 1) * 128, half * 1024 : (half + 1) * 1024],
                        ob[:],
                    )
    nc.finalize()
    return nc


def _get_nc():
    if "nc" not in _CACHE:
        _CACHE["nc"] = build_nc()
    return _CACHE["nc"]


def kernel(x, Wqkv, bqkv, Wout, bout):
    x = np.asarray(x, np.float32)
    Wqkv = np.asarray(Wqkv, np.float32)
    bqkv = np.asarray(bqkv, np.float32)
    Wout = np.asarray(Wout, np.float32)
    bout = np.asarray(bout, np.float32)
    B = x.shape[0]
    ident = np.eye(128, dtype=np.float32)
    ones_row16 = np.ones((1, N_TOK), np.float16)

    in_maps = []
    for c in range(8):
        b, g = c // 4, c % 4
        xt = np.concatenate(
            [np.ascontiguousarray(x[b].T).astype(np.float16), ones_row16], 0
        )
        cols, bias = [], []
        for blk in range(3):  # q, k, v column blocks of Wqkv
            s = blk * DIM + g * NH * DH
            cols.append(Wqkv[:, s : s + NH * DH])
            bias.append(bqkv[s : s + NH * DH])
        wg = np.concatenate(
            [np.concatenate(cols, 1), np.concatenate(bias)[None, :]], 0
        ).astype(np.float16)
        qb = bqkv[g * 256 : (g + 1) * 256]
        kb = bqkv[DIM + g * 256 : DIM + (g + 1) * 256]
        qkb = np.stack([qb[:128], qb[128:], kb[:128], kb[128:]], 1).astype(np.float32)
        wo = np.ascontiguousarray(Wout[g * NH * DH : (g + 1) * NH * DH, :]).astype(
            np.float16
        )
        in_maps.append(
            {
                "xt": np.ascontiguousarray(xt),
                "wg": np.ascontiguousarray(wg),
                "qkb": np.ascontiguousarray(qkb),
                "wout": wo,
                "ident": ident,
                "ones16": np.ones((1, DH), np.float16),
            }
        )

    _CACHE["last_in_maps"] = in_maps
    res = run_bass_kernel_spmd(_get_nc(), in_maps, list(range(8))).results
    out = np.empty((B, N_TOK, DIM), np.float32)
    for b in range(B):
        acc = res[4 * b]["out"].astype(np.float32)
        for g in range(1, 4):
            acc = acc + res[4 * b + g]["out"].astype(np.float32)
        out[b] = acc.T + bout[None, :]
    return out


if __name__ == "__main__":
    rng = np.random.default_rng(0)
    x = rng.standard_normal((2, N_TOK, DIM)).astype(np.float32)
    Wqkv = (rng.standard_normal((DIM, 3 * DIM)) * DIM**-0.5).astype(np.float32)
    bqkv = (rng.standard_normal(3 * DIM) * 0.02).astype(np.float32)
    Wout = (rng.standard_normal((DIM, DIM)) * DIM**-0.5).astype(np.float32)
    bout = (rng.standard_normal(DIM) * 0.02).astype(np.float32)
    o = kernel(x=x, Wqkv=Wqkv, bqkv=bqkv, Wout=Wout, bout=bout)
    print("kernel ran, out shape", o.shape)


# revision 25
# speedup vs baseline: 1.2000x; 1.0402x over previous
"""Trn2 Bass kernel for nn_Attention_16793322128104.

Sharding: 8 cores = 2 batches x 4 head-groups (4 heads each).
Per core: fp16 QKV projection (768 Wqkv cols), 4 attention heads with
exact per-query max (pass-1 fp16 S + vector reduce_max), softmax shift
folded into the S^T matmul as a 65th contraction row (fp16), exp on the
scalar engine into an fp16 PT ring, AV with ones-column denominator,
deferred batched normalization, fp16 partial out-projection (host sums
the 4 head-group partials per batch).

Pipeline: head h's S^T/exp overlaps AV of head h-1 (PT ring, 24 slots)
and pass-1 of head h+1. PSUM: 3 rotating [128,1024] stream slots +
a 2-bank AV accumulator.
"""

import sys
from contextlib import ExitStack

import numpy as np

sys.path.insert(0, "/opt/trn_rl_repo")

import concourse.bass as bass
import concourse.bacc as bacc
import concourse.mybir as mybir
from concourse import tile
from concourse.bass_utils import run_bass_kernel_spmd

F32 = mybir.dt.float32
F16 = mybir.dt.float16
IDENT = mybir.ActivationFunctionType.Identity
EXP = mybir.ActivationFunctionType.Exp
XAX = mybir.AxisListType.X

N_TOK = 2048
DIM = 1024
NH = 4                # heads per core
DH = 64               # head dim
SCALE = 8.0           # sqrt(DH); reference MULTIPLIES by sqrt(d_head)
RING = 24             # PT ring slots (16 per head, AV trails one head)

_CACHE = {}


def _rs(h, m):
    return (16 * h + m) % RING


def build_nc():
    nc = bacc.Bacc()
    xt_d = nc.declare_dram_parameter("xt", [DIM + 1, N_TOK], F16, isOutput=False)
    wg_d = nc.declare_dram_parameter("wg", [DIM + 1, 3 * NH * DH], F16, isOutput=False)
    qkb_d = nc.declare_dram_parameter("qkb", [128, 4], F32, isOutput=False)
    wo_d = nc.declare_dram_parameter("wout", [2 * 128, DIM], F16, isOutput=False)
    id_d = nc.declare_dram_parameter("ident", [128, 128], F32, isOutput=False)
    on16_d = nc.declare_dram_parameter("ones16", [1, DH], F16, isOutput=False)
    out_d = nc.declare_dram_parameter("out", [DIM, N_TOK], F16, isOutput=True)

    with ExitStack() as ctx:
        tc = ctx.enter_context(tile.TileContext(nc))
        pers = ctx.enter_context(tc.tile_pool(name="pers", bufs=1))
        ps = ctx.enter_context(
            tc.tile_pool(name="ps", bufs=3, space=bass.MemorySpace.PSUM)
        )
        pop = ctx.enter_context(
            tc.tile_pool(name="pop", bufs=1, space=bass.MemorySpace.PSUM)
        )

        q2 = [pers.tile([DH + 1, N_TOK], F16, tag=f"q2{h}", name=f"q2{h}") for h in range(NH)]
        k2 = [pers.tile([DH + 1, N_TOK], F16, tag=f"k2{h}", name=f"k2{h}") for h in range(NH)]
        vsb = [pers.tile([128, NH, DH + 1], F16, tag=f"v{m}", name=f"v{m}") for m in range(16)]
        o2 = [pers.tile([128, N_TOK], F16, tag=f"o2{t}", name=f"o2{t}") for t in range(2)]
        wo_sb = [pers.tile([128, DIM], F16, tag=f"wo{t}", name=f"wo{t}") for t in range(2)]
        ident = pers.tile([128, 128], F32, tag="id", name="identsb")
        qkb = pers.tile([128, 4], F32, tag="qkb", name="qkbsb")
        on16 = pers.tile([1, DH], F16, tag="on16", name="on16sb")
        mc2 = [pers.tile([128, 16, 2], F32, tag=f"mc{h}", name=f"mc{h}") for h in range(NH)]
        mcf = [pers.tile([128, 16], F32, tag=f"mcf{h}", name=f"mcf{h}") for h in range(NH)]
        negm = [
            [pers.tile([8, 128], F16, tag=f"nm{h}{u}", name=f"nm{h}{u}") for u in range(2)]
            for h in range(NH)
        ]
        # deferred-normalization temporaries
        denall = pers.tile([8, N_TOK // 2], F16, tag="denall", name="denall")
        dtmp = [pers.tile([1, N_TOK // 2], F16, tag=f"dtmp{p}", name=f"dtmp{p}") for p in range(2)]
        dnf = [pers.tile([128, 8], F16, tag=f"dnf{p}", name=f"dnf{p}") for p in range(2)]
        dnr = [pers.tile([128, 8], F32, tag=f"dnr{p}", name=f"dnr{p}") for p in range(2)]
        dnh = [pers.tile([128, 8], F16, tag=f"dnh{p}", name=f"dnh{p}") for p in range(2)]
        rrj = [pers.tile([1, N_TOK // 2], F16, tag=f"rr{p}", name=f"rr{p}") for p in range(2)]
        rm = pers.tile([128, N_TOK // 2], F16, tag="rm", name="rmsb")

        nc.sync.dma_start(ident[:], id_d[:])
        nc.sync.dma_start(qkb[:], qkb_d[:])
        nc.sync.dma_start(on16[:], on16_d[:])
        for h in range(NH):
            nc.sync.dma_start(k2[h][DH : DH + 1, :], xt_d[DIM : DIM + 1, :])
        for m in range(16):
            nc.vector.memset(vsb[m][:, :, DH : DH + 1], 1.0)
        for t in range(2):
            nc.sync.dma_start(wo_sb[t][:], wo_d[t * 128 : (t + 1) * 128, :])

        # ---------------- pass-1 helper: per-query max for head h ----------
        def negm_half(h, half):
            # fold maxes for qt block [8*half, 8*half+8) into q2 row 64
            qs = slice(8 * half, 8 * half + 8)
            nc.vector.reduce_max(mcf[h][:, qs], mc2[h][:, qs, :], axis=XAX)
            pst = ps.tile([128, N_TOK // 2], F32, tag="s", name="pst")
            nc.tensor.transpose(pst[0:8, 0:128], mcf[h][:, qs], ident[:])
            nc.vector.tensor_scalar_mul(negm[h][half][:], pst[0:8, 0:128], -1.0)
            nc.sync.dma_start(
                q2[h][DH : DH + 1, half * 1024 : (half + 1) * 1024], negm[h][half][:]
            )

        def p1_qt(h, qt):
            for half in range(2):
                p = ps.tile([128, N_TOK // 2], F32, tag="s", name="p1")
                for kc in range(2):
                    nc.tensor.matmul(
                        p[:, kc * 512 : (kc + 1) * 512],
                        q2[h][0:DH, qt * 128 : (qt + 1) * 128],
                        k2[h][0:DH, half * 1024 + kc * 512 : half * 1024 + (kc + 1) * 512],
                        start=True,
                        stop=True,
                    )
                nc.vector.reduce_max(mc2[h][:, qt, half : half + 1], p[:], axis=XAX)
            if qt == 8:
                negm_half(h, 0)
            if qt == 15:
                negm_half(h, 1)

        # ---------------- AV + deferred normalization helpers --------------
        def av_j_mm(h, j, m2, po):
            for u in range(2):
                nc.tensor.matmul(
                    po[:, u * 512 : (u + 1) * 512],
                    vsb[m2][:, h, :],
                    PT[:, _rs(h, m2), j * 1024 + u * 512 : j * 1024 + (u + 1) * 512],
                    start=(m2 == 0),
                    stop=(m2 == 15),
                )

        def o2q(h, j):
            return o2[h // 2][(h % 2) * DH : (h % 2) * DH + DH, j * 1024 : (j + 1) * 1024]

        def stash_j(h, j, po):
            # raw (unnormalized) AV output into o2; denominator row stashed
            # via a partition-0 ping tile. Normalized in a batched tail pass.
            idx = 2 * h + j
            p = idx % 2
            nc.vector.tensor_copy(o2q(h, j), po[0:DH, :])
            nc.scalar.copy(dtmp[p][:], po[DH : DH + 1, :])
            nc.sync.dma_start(denall[idx : idx + 1, :], dtmp[p][:])

        def norm_j(h, j):
            p = j % 2
            idx = 2 * h + j
            nc.sync.dma_start(dnf[p][:], denall[idx : idx + 1, :])
            nc.vector.reciprocal(dnr[p][:], dnf[p][:])
            nc.vector.tensor_copy(dnh[p][:], dnr[p][:])
            nc.sync.dma_start(rrj[p][:], dnh[p][:])
            pr = ps.tile([128, N_TOK // 2], F32, tag="s", name="pr")
            for u in range(2):
                nc.tensor.matmul(
                    pr[0:DH, u * 512 : (u + 1) * 512],
                    on16[:], rrj[p][:, u * 512 : (u + 1) * 512],
                    start=True, stop=True,
                )
            rows = slice((h % 2) * DH, (h % 2) * DH + DH)
            nc.scalar.copy(rm[rows, :], pr[0:DH, :])
            nc.vector.tensor_mul(o2q(h, j), o2q(h, j), rm[rows, :])

        def av_step(h, m, po_box):
            # two AV m2-tiles of head h at pipeline step m (j = m // 8)
            j = m // 8
            if m % 8 == 0:
                if m == 8:
                    stash_j(h, 0, po_box[0])
                po_box[0] = pop.tile([DH + 1, N_TOK // 2], F32, tag="po", name="po")
            av_j_mm(h, j, 2 * (m % 8), po_box[0])
            av_j_mm(h, j, 2 * (m % 8) + 1, po_box[0])

        # ---------------- phase A: QKV projection (fp16) -------------------
        with (
            tc.tile_pool(name="xt", bufs=1) as xt_pool,
            tc.tile_pool(name="wgp", bufs=1) as wg_pool,
        ):
            xt_sb = xt_pool.tile([128, 8, N_TOK], F16, tag="xta", name="xta")
            xtr = xt_pool.tile([1, N_TOK], F16, tag="xtr", name="xtr")
            wg_sb = wg_pool.tile([128, 8, 3 * NH * DH], F16, tag="wga", name="wga")
            wgr = wg_pool.tile([1, 3 * NH * DH], F16, tag="wgr", name="wgr")
            for c in range(8):
                nc.sync.dma_start(wg_sb[:, c, :], wg_d[c * 128 : (c + 1) * 128, :])
                nc.sync.dma_start(xt_sb[:, c, :], xt_d[c * 128 : (c + 1) * 128, :])
            nc.sync.dma_start(xtr[:], xt_d[DIM : DIM + 1, :])
            nc.sync.dma_start(wgr[:], wg_d[DIM : DIM + 1, :])

            # q/k waves: ft 0/1 = q heads01/23 (cols 0:256), 2/3 = k (256:512)
            def qk_wave(ft):
                dst = q2 if ft < 2 else k2
                hb = 2 * (ft % 2)
                pw = [ps.tile([128, N_TOK // 2], F32, tag="s", name="pw") for _ in range(2)]
                for c in range(8):
                    for half in range(2):
                        for tj in range(2):
                            nc.tensor.matmul(
                                pw[half][:, tj * 512 : (tj + 1) * 512],
                                wg_sb[:, c, ft * 128 : (ft + 1) * 128],
                                xt_sb[:, c, half * 1024 + tj * 512 : half * 1024 + (tj + 1) * 512],
                                start=(c == 0),
                                stop=(c == 7),
                            )
                for half in range(2):
                    cols = slice(half * 1024, (half + 1) * 1024)
                    nc.scalar.activation(
                        dst[hb][0:DH, cols], pw[half][0:DH, :], IDENT,
                        bias=qkb[0:DH, ft : ft + 1],
                    )
                    nc.scalar.activation(
                        dst[hb + 1][0:DH, cols], pw[half][DH:128, :], IDENT,
                        bias=qkb[DH:128, ft : ft + 1],
                    )

            def v_tile(i):
                # two token-tiles (2i, 2i+1) of v into one psum tile
                pv = ps.tile([128, N_TOK // 2], F32, tag="s", name="pv")
                for c in range(9):
                    for u in range(2):
                        tt = 2 * i + u
                        lhs = (
                            xt_sb[:, c, tt * 128 : (tt + 1) * 128]
                            if c < 8
                            else xtr[:, tt * 128 : (tt + 1) * 128]
                        )
                        rhs = (
                            wg_sb[:, c, 2 * NH * DH : 3 * NH * DH]
                            if c < 8
                            else wgr[:, 2 * NH * DH : 3 * NH * DH]
                        )
                        nc.tensor.matmul(
                            pv[:, u * 512 : u * 512 + NH * DH],
                            lhs, rhs,
                            start=(c == 0),
                            stop=(c == 8),
                        )
                for u in range(2):
                    tt = 2 * i + u
                    nc.scalar.copy(
                        vsb[tt][:, :, 0:DH],
                        pv[:, u * 512 : u * 512 + NH * DH].rearrange(
                            "p (h d) -> p h d", h=NH
                        ),
                    )

            qk_wave(0)          # q heads 0,1
            qk_wave(2)          # k heads 0,1
            # interleave q23/k23 waves and v tiles with pass-1 of head 0
            qk_wave(1)
            p1_qt(0, 0)
            qk_wave(3)
            p1_qt(0, 1)
            for i in range(8):
                v_tile(i)
                if i < 7:
                    p1_qt(0, 2 + 2 * i)
                    p1_qt(0, 3 + 2 * i)

        # ---------------- attention: S^T + exp, pipelined ------------------
        # PT ring lives in its own pool so it reuses the SBUF freed by the
        # xt/wg pools above.
        ptp = ctx.enter_context(tc.tile_pool(name="ptp", bufs=1))
        PT = ptp.tile([128, RING, N_TOK], F16, tag="PT", name="PTsb")

        # pass-1 qts per pipeline step: 2 at steps 0/1, last at step 13, so
        # the negm flush lands 2 steps before the next pp needs q2 row 64.
        P1_SCHED = {0: [0, 1], 1: [2, 3]}
        for _s in range(2, 14):
            P1_SCHED[_s] = [_s + 2]

        def pp(h):
            hp = h + 1 if h < 3 else None     # pass-1 head piggybacked
            ha = h - 1 if h >= 1 else None    # AV head piggybacked
            po_box = [None]
            for m in range(16):
                # AV for h-1 first: exp(h, m) reuses the PT ring slot that
                # AV(h-1) reads at this step, so its readers must be emitted
                # ahead of the overwrite.
                if ha is not None:
                    av_step(ha, m, po_box)
                for half in range(2):
                    st = ps.tile([128, N_TOK // 2], F32, tag="s", name="st")
                    for j2 in range(2):
                        nc.tensor.matmul(
                            st[:, j2 * 512 : (j2 + 1) * 512],
                            k2[h][:, m * 128 : (m + 1) * 128],
                            q2[h][:, half * 1024 + j2 * 512 : half * 1024 + (j2 + 1) * 512],
                            start=True,
                            stop=True,
                        )
                    nc.scalar.activation(
                        PT[:, _rs(h, m), half * 1024 : (half + 1) * 1024],
                        st[:], EXP, scale=SCALE,
                    )
                if hp is not None:
                    for qt in P1_SCHED.get(m, []):
                        p1_qt(hp, qt)
            if ha is not None:
                stash_j(ha, 1, po_box[0])

        for h in range(NH):
            pp(h)

        # tail: AV(3) with norms interleaved; out projection per token-half
        # starts as soon as that half's norms are done.
        with tc.tile_pool(name="ob", bufs=3) as obp:
            def outproj(dc, half):
                pout = ps.tile([128, N_TOK // 2], F32, tag="s", name="pout")
                for ht in range(2):
                    for u in range(2):
                        nc.tensor.matmul(
                            pout[:, u * 512 : (u + 1) * 512],
                            wo_sb[ht][:, dc * 128 : (dc + 1) * 128],
                            o2[ht][:, half * 1024 + u * 512 : half * 1024 + (u + 1) * 512],
                            start=(ht == 0),
                            stop=(ht == 1),
                        )
                ob = obp.tile([128, N_TOK // 2], F16, tag="ob", name="ob")
                if (2 * dc + half) % 2 == 0:
                    nc.scalar.copy(ob[:], pout[:])
                else:
                    nc.vector.tensor_copy(ob[:], pout[:])
                nc.sync.dma_start(
                    out_d[dc * 128 : (dc + 1) * 128, half * 1024 : (half + 1) * 1024],
                    ob[:],
                )

            J0N = {2: (0, 0), 5: (1, 0), 7: (2, 0)}
            J1N = {8: (3, 0), 9: (0, 1), 11: (1, 1), 13: (2, 1)}
            po_box = [None]
            for m in range(16):
                av_step(3, m, po_box)  # stashes (3,0) at m==8
                if m in J0N:
                    norm_j(*J0N[m])
                if m in J1N:
                    norm_j(*J1N[m])
                if 10 <= m <= 13:  # half-0 outproj once its norms are done
                    outproj(2 * (m - 10), 0)
                    outproj(2 * (m - 10) + 1, 0)
            stash_j(3, 1, po_box[0])
            norm_j(3, 1)
            for dc in range(8):
                outproj(dc, 1)
    nc.finalize()
    return nc


def _get_nc():
    if "nc" not in _CACHE:
        _CACHE["nc"] = build_nc()
    return _CACHE["nc"]


def kernel(x, Wqkv, bqkv, Wout, bout):
    x = np.asarray(x, np.float32)
    Wqkv = np.asarray(Wqkv, np.float32)
    bqkv = np.asarray(bqkv, np.float32)
    Wout = np.asarray(Wout, np.float32)
    bout = np.asarray(bout, np.float32)
    B = x.shape[0]
    ident = np.eye(128, dtype=np.float32)
    ones_row16 = np.ones((1, N_TOK), np.float16)

    in_maps = []
    for c in range(8):
        b, g = c // 4, c % 4
        xt = np.concatenate(
            [np.ascontiguousarray(x[b].T).astype(np.float16), ones_row16], 0
        )
        cols, bias = [], []
        for blk in range(3):  # q, k, v column blocks of Wqkv
            s = blk * DIM + g * NH * DH
            cols.append(Wqkv[:, s : s + NH * DH])
            bias.append(bqkv[s : s + NH * DH])
        wg = np.concatenate(
            [np.concatenate(cols, 1), np.concatenate(bias)[None, :]], 0
        ).astype(np.float16)
        qb = bqkv[g * 256 : (g + 1) * 256]
        kb = bqkv[DIM + g * 256 : DIM + (g + 1) * 256]
        qkb = np.stack([qb[:128], qb[128:], kb[:128], kb[128:]], 1).astype(np.float32)
        wo = np.ascontiguousarray(Wout[g * NH * DH : (g + 1) * NH * DH, :]).astype(
            np.float16
        )
        in_maps.append(
            {
                "xt": np.ascontiguousarray(xt),
                "wg": np.ascontiguousarray(wg),
                "qkb": np.ascontiguousarray(qkb),
                "wout": wo,
                "ident": ident,
                "ones16": np.ones((1, DH), np.float16),
            }
        )

    _CACHE["last_in_maps"] = in_maps
    res = run_bass_kernel_spmd(_get_nc(), in_maps, list(range(8))).results
    out = np.empty((B, N_TOK, DIM), np.float32)
    for b in range(B):
        acc = res[4 * b]["out"].astype(np.float32)
        for g in range(1, 4):
            acc = acc + res[4 * b + g]["out"].astype(np.float32)
        out[b] = acc.T + bout[None, :]
    return out


if __name__ == "__main__":
    rng = np.random.default_rng(0)
    x = rng.standard_normal((2, N_TOK, DIM)).astype(np.float32)
    Wqkv = (rng.standard_normal((DIM, 3 * DIM)) * DIM**-0.5).astype(np.float32)
    bqkv = (rng.standard_normal(3 * DIM) * 0.02).astype(np.float32)
    Wout = (rng.standard_normal((DIM, DIM)) * DIM**-0.5).astype(np.float32)
    bout = (rng.standard_normal(DIM) * 0.02).astype(np.float32)
    o = kernel(x=x, Wqkv=Wqkv, bqkv=bqkv, Wout=Wout, bout=bout)
    print("kernel ran, out shape", o.shape)
